# revision 1
# baseline (speedup 1.0000x reference)
"""Trainium2 Bass kernel for nn_Cross_Frequency_Enhanced_Block.

kernel(**inputs) takes FULL unsharded inputs (as in setup_inputs()) and
returns the FULL (32, 1024, 512) float32 output.

Sharding: data-parallel over batch B across 8 NeuronCores (4 batches/core).

Algorithm notes (validated vs reference in numpy, absmax ~4e-6):
  - rfft(x @ Wq.T)[:, :32] == Wq @ rfft(x)[:, :32]: DFT x once per batch via
    matmuls against cos/sin tables (only 32 modes needed), apply Wq/Wk in the
    frequency domain; q/k never materialize in the time domain.
  - complex tanh via the stable sech formula with Cody-Waite range reduction
    for sin/cos (ACT Sin domain is [-pi, pi]).
  - per-mode complex weight einsum: stationary [VR|VI] / [-VI|VR] column
    pairs, moving = w mode-slab (f32r, N=512).
  - irfft as matmul against a (64, 1024) table (1/(D*D) and 2/L folded in).
  - moving average (k=128, edge replicate) via DVE prefix scan + shifted
    differences.  u - mov(u) kills any constant bias exactly, so bo drops.
  - BatchNorm(eval) folded into the final PE-transpose eviction as per-l
    scale/bias on ACT.

Matmul operands are float32r end-to-end (DRAM->SBUF->PE); walrus requires
f32r consumers to see f32r producers.  All engine (DVE/ACT) multi-operand
ops keep every operand at the same start partition.
"""

import os
from contextlib import ExitStack

import numpy as np

import concourse.bacc as bacc
import concourse.bass as bass
import concourse.tile as tile
import concourse.mybir as mybir
from concourse.bass_utils import run_bass_kernel_spmd

B, L, D, MODES = 32, 1024, 512, 32
NCORES = 8
BPC = B // NCORES
F32 = mybir.dt.float32
FR = mybir.dt.float32r
AF = mybir.ActivationFunctionType
ALU = mybir.AluOpType

MAGIC = float(np.float32(12582912.0))        # 1.5*2^23 round-to-nearest
CW1 = float(np.float32(6.28125))             # 2pi hi (exact in f32)
CW2 = float(2 * np.pi - 6.28125)             # 2pi lo
INV2PI = float(np.float32(1.0 / (2 * np.pi)))
PI = float(np.float32(np.pi))
SIM_GELU = bool(int(os.environ.get("BK_SIM_GELU", "0")))


def _tables():
    l_ = np.arange(L)[:, None].astype(np.float64)
    m_ = np.arange(MODES)[None, :].astype(np.float64)
    ang = 2 * np.pi * l_ * m_ / L
    F = np.concatenate([np.cos(ang), -np.sin(ang)], 1).astype(np.float32)
    ftab = np.ascontiguousarray(F.reshape(8, 128, 64).transpose(1, 0, 2))

    a = np.full((MODES,), 2.0 / L)
    a[0] = 1.0 / L
    a = a / (D * D)
    Gc = a[:, None] * np.cos(2 * np.pi * m_.T * l_.T / L)
    Gs = a[:, None] * -np.sin(2 * np.pi * m_.T * l_.T / L)
    gtab = np.concatenate([Gc, Gs], 0).astype(np.float32)

    ident = np.eye(128, dtype=np.float32)
    coefF = np.ascontiguousarray(np.broadcast_to(
        64.0 - np.arange(65, dtype=np.float32), (128, 65)))
    coefE = np.ascontiguousarray(np.broadcast_to(
        np.arange(63, dtype=np.float32) + 1.0, (128, 63)))
    return ftab, gtab, ident, coefF, coefE


def _t128(w):
    """(512, 512) host array -> (128, 4, 512) [p, ch, col] with row=ch*128+p."""
    return np.ascontiguousarray(w.reshape(4, 128, 512).transpose(1, 0, 2))


def _build():
    nc = bacc.Bacc("TRN2", target_bir_lowering=False, debug=False,
                   num_devices=NCORES)
    dram = {}

    def din(name, shape, dt=FR):
        dram[name] = nc.dram_tensor(name, list(shape), dt,
                                    kind="ExternalInput").ap()

    MLOC = MODES // NCORES               # modes owned per core
    din("xs", (BPC, L, D))
    din("wslab", (MLOC, 2, D, D))        # per-core mode slice of w
    for n in ("wqt", "wkt", "wot", "w1t", "w2t"):
        din(n, (128, 4, D))
    din("ftab", (128, 8, 64))
    din("gtab", (64, L))
    din("identt", (128, 128))
    din("coefF", (128, 65))
    din("coefE", (128, 63))
    din("bnt", (128, 8, 4), F32)
    din("bqkt", (128, 4, 2), F32)
    out_d = nc.dram_tensor("out", [BPC, L, D], F32, kind="ExternalOutput").ap()
    # collective staging: xqkv -> mode owners, einsum result -> batch owners
    vq_d = nc.dram_tensor("vq_d", [BPC, 64, D], FR).ap()
    vq_snd = nc.dram_tensor("vq_snd", [NCORES, BPC, 2, MLOC, D], FR).ap()
    vq_rcv = nc.dram_tensor("vq_rcv", [NCORES, BPC, 2, MLOC, D], FR).ap()
    xwm_d = nc.dram_tensor("xwm_d", [MLOC, 64, D], FR).ap()
    xw_snd = nc.dram_tensor("xw_snd", [NCORES, MLOC, 2, BPC, D], FR).ap()
    xw_rcv = nc.dram_tensor("xw_rcv", [NCORES, MLOC, 2, BPC, D], FR).ap()

    with tile.TileContext(nc) as tc, ExitStack() as ctx:
        con = ctx.enter_context(tc.tile_pool(name="con", bufs=1))
        wrk = ctx.enter_context(tc.tile_pool(name="wrk", bufs=1))
        big = ctx.enter_context(tc.tile_pool(name="big", bufs=1))
        wpool = ctx.enter_context(tc.tile_pool(name="wpool", bufs=6))
        outp = ctx.enter_context(tc.tile_pool(name="outp", bufs=2))
        ps = ctx.enter_context(tc.tile_pool(name="ps", bufs=4, space="PSUM"))
        ps2 = ctx.enter_context(tc.tile_pool(name="ps2", bufs=2, space="PSUM"))

        def cload(name, shape, dt=FR):
            t = con.tile(list(shape), dt, tag=name)
            nc.sync.dma_start(out=t[:], in_=dram[name])
            return t

        ftab = cload("ftab", (128, 8, 64))
        gtab = cload("gtab", (64, L))
        ident = cload("identt", (128, 128))
        wqt = cload("wqt", (128, 4, D))
        wkt = cload("wkt", (128, 4, D))
        wot = cload("wot", (128, 4, D))
        w1t = cload("w1t", (128, 4, D))
        w2t = cload("w2t", (128, 4, D))
        coefF = cload("coefF", (128, 65))
        coefE = cload("coefE", (128, 63))
        bnt = cload("bnt", (128, 8, 4), F32)
        bqkt = cload("bqkt", (128, 4, 2), F32)

        def tt(o, a, bb, op):
            nc.vector.tensor_tensor(o, a, bb, op)

        # --- BN constants: c1 = gamma*rsqrt(var+eps), c0 = beta - mean*c1
        c1l = con.tile([128, 8], F32)
        c0l = con.tile([128, 8], F32)
        sq = con.tile([128, 8], F32)
        ve = con.tile([128, 8], F32)
        yy = con.tile([128, 8], F32)
        nc.vector.tensor_scalar_add(ve[:], bnt[:, :, 3], 1e-5)
        nc.scalar.activation(sq[:], ve[:], AF.Sqrt)
        nc.vector.reciprocal(c1l[:], sq[:])
        tt(yy[:], c1l[:], c1l[:], ALU.mult)
        tt(yy[:], yy[:], ve[:], ALU.mult)
        nc.vector.tensor_scalar(yy[:], yy[:], -0.5, 1.5, ALU.mult, ALU.add)
        tt(c1l[:], c1l[:], yy[:], ALU.mult)
        tt(c1l[:], c1l[:], bnt[:, :, 0], ALU.mult)
        tt(c0l[:], bnt[:, :, 2], c1l[:], ALU.mult)
        tt(c0l[:], bnt[:, :, 1], c0l[:], ALU.subtract)
        bqs = con.tile([128, 4, 2], FR)
        nc.scalar.activation(bqs[:], bqkt[:], AF.Copy, scale=float(L))

        # =================== stage A: x load + 32-mode DFT =================
        xf_all = wrk.tile([128, 4, BPC, 64], FR, tag="mid2")
        for b in range(BPC):
            xt = big.tile([128, 8, D], FR, tag="xx")
            nc.sync.dma_start(
                out=xt[:],
                in_=dram["xs"][b].rearrange("(t p) d -> p t d", p=128))
            xfT_ps = ps.tile([128, 512], F32, tag="ps")
            for lt in range(8):
                nc.tensor.matmul(xfT_ps[0:64, :], ftab[:, lt, :],
                                 xt[:, lt, :],
                                 start=(lt == 0), stop=(lt == 7))
            xfT_sb = wrk.tile([64, 512], FR, tag="xfT")
            nc.vector.tensor_copy(xfT_sb[:], xfT_ps[0:64, :])
            xf_ps = ps.tile([128, 512], FR, tag="ps")
            for dch in range(4):
                nc.tensor.transpose(xf_ps[:, dch * 64:dch * 64 + 64],
                                    xfT_sb[:, dch * 128:dch * 128 + 128],
                                    ident[0:64, 0:64])
            nc.vector.tensor_copy(
                xf_all[:, :, b, :],
                xf_ps[:, 0:256].rearrange("p (c m) -> p c m", c=4))

        # =================== stage B: qf/kf in frequency domain ============
        qkf = wrk.tile([128, 4, BPC, 128], FR, tag="mid")
        for wt, co in ((wqt, 0), (wkt, 64)):
            for ech in range(4):
                qp = ps.tile([128, 512], F32, tag="ps")
                for dch in range(4):
                    nc.tensor.matmul(
                        qp[:, 0:256], wt[:, dch, ech * 128:ech * 128 + 128],
                        xf_all[:, dch, :, :],
                        start=(dch == 0), stop=(dch == 3))
                nc.vector.tensor_copy(
                    qkf[:, ech, :, co:co + 64],
                    qp[:, 0:256].rearrange("p (b m) -> p b m", b=BPC))
        # bias: mode-0 real += L*b  (DFT of constant vector)
        for ech in range(4):
            for co, j in ((0, 0), (64, 1)):
                tt(qkf[:, ech, :, co:co + 1], qkf[:, ech, :, co:co + 1],
                   bqs[:, ech:ech + 1, j:j + 1].to_broadcast([128, BPC, 1]),
                   ALU.add)

        # =================== stage C: Z, tanh, U, xqkv =====================
        # Z split into Re/Im row blocks so every engine op stays at start
        # partition 0:  ZR[x, y'] = sum_e qfRe[e, x] kf[e, y'], ZI likewise.
        ZpsR = ps.tile([32, 512], F32, tag="ps")
        ZpsI = ps.tile([32, 512], F32, tag="ps")
        for b in range(BPC):
            for ech in range(4):
                nc.tensor.matmul(
                    ZpsR[0:32, b * 64:b * 64 + 64],
                    qkf[:, ech, b, 0:32], qkf[:, ech, b, 64:128],
                    start=(ech == 0), stop=(ech == 3))
                nc.tensor.matmul(
                    ZpsI[0:32, b * 64:b * 64 + 64],
                    qkf[:, ech, b, 32:64], qkf[:, ech, b, 64:128],
                    start=(ech == 0), stop=(ech == 3))
        ZsbR = wrk.tile([32, BPC, 64], F32)
        ZsbI = wrk.tile([32, BPC, 64], F32)
        nc.vector.tensor_copy(
            ZsbR[:], ZpsR[0:32, 0:256].rearrange("p (b y) -> p b y", b=BPC))
        nc.vector.tensor_copy(
            ZsbI[:], ZpsI[0:32, 0:256].rearrange("p (b y) -> p b y", b=BPC))

        sh = [32, BPC, 32]
        zr = wrk.tile(sh, F32)
        zi = wrk.tile(sh, F32)
        # Z = (QR + iQI).(KR + iKI):  Re = QR.KR - QI.KI, Im = QR.KI + QI.KR
        tt(zr[:], ZsbR[:, :, 0:32], ZsbI[:, :, 32:64], ALU.subtract)
        tt(zi[:], ZsbR[:, :, 32:64], ZsbI[:, :, 0:32], ALU.add)
        tht = wrk.tile(sh, F32)
        sech = wrk.tile(sh, F32)
        s2y = wrk.tile(sh, F32)
        c2y = wrk.tile(sh, F32)
        w1 = wrk.tile(sh, F32)
        w2 = wrk.tile(sh, F32)
        w3 = wrk.tile(sh, F32)
        nc.scalar.activation(tht[:], zr[:], AF.Tanh, scale=2.0)
        nc.scalar.activation(w1[:], zr[:], AF.Abs, scale=2.0)
        nc.vector.tensor_scalar_min(w1[:], w1[:], 87.0)
        nc.scalar.activation(w1[:], w1[:], AF.Exp, scale=-1.0)   # e^-2|x|
        tt(w2[:], w1[:], w1[:], ALU.mult)
        nc.vector.tensor_scalar_add(w2[:], w2[:], 1.0)
        nc.vector.reciprocal(w2[:], w2[:])
        tt(sech[:], w1[:], w2[:], ALU.mult)
        nc.vector.tensor_scalar(sech[:], sech[:], 2.0, None, ALU.mult)
        for dst, ofs in ((s2y, 0.0), (c2y, 0.25)):
            # k = round(2*zi/(2pi) + ofs) via the 1.5*2^23 magic-add trick
            nc.vector.tensor_scalar(w1[:], zi[:], 2.0 * INV2PI, MAGIC + ofs,
                                    ALU.mult, ALU.add)
            nc.vector.tensor_scalar_sub(w1[:], w1[:], MAGIC)
            # red = 2*zi (+ pi/2 for cos) - k*CW1 - k*CW2, clamp to [-pi, pi]
            nc.vector.tensor_scalar(w2[:], zi[:], 2.0, ofs * 2.0 * PI,
                                    ALU.mult, ALU.add)
            nc.vector.tensor_scalar(w3[:], w1[:], CW1, None, ALU.mult)
            tt(w2[:], w2[:], w3[:], ALU.subtract)
            nc.vector.tensor_scalar(w3[:], w1[:], CW2, None, ALU.mult)
            tt(w2[:], w2[:], w3[:], ALU.subtract)
            nc.vector.tensor_scalar(w2[:], w2[:], -PI, PI, ALU.max, ALU.min)
            nc.scalar.activation(dst[:], w2[:], AF.Sin)
        tt(w1[:], c2y[:], sech[:], ALU.mult)
        nc.vector.tensor_scalar_add(w1[:], w1[:], 1.0)
        nc.vector.reciprocal(w1[:], w1[:])                       # 1/den
        TR = wrk.tile(sh, FR)
        TI = wrk.tile(sh, FR)
        tt(TR[:], tht[:], w1[:], ALU.mult)
        tt(TI[:], s2y[:], sech[:], ALU.mult)
        tt(TI[:], TI[:], w1[:], ALU.mult)
        # U1 = [TR^T | TI^T], U2 = [-TI^T | TR^T] per batch (start part 0)
        U1 = wrk.tile([32, BPC, 64], FR)
        U2 = wrk.tile([32, BPC, 64], FR)
        for b in range(BPC):
            tp1 = ps.tile([32, 128], FR, tag="ps")
            nc.tensor.transpose(tp1[0:32, 0:32], TR[:, b, :],
                                ident[0:32, 0:32])
            nc.tensor.transpose(tp1[0:32, 64:96], TI[:, b, :],
                                ident[0:32, 0:32])
            nc.vector.tensor_copy(U1[:, b, 0:32], tp1[0:32, 0:32])
            nc.vector.tensor_copy(U1[:, b, 32:64], tp1[0:32, 64:96])
            nc.scalar.activation(U2[:, b, 0:32], tp1[0:32, 64:96],
                                 AF.Copy, scale=-1.0)
            nc.vector.tensor_copy(U2[:, b, 32:64], tp1[0:32, 0:32])
        # xqkv per b -> transpose to (col, e) and stage to DRAM for AllToAll
        for b in range(BPC):
            kpsR = ps.tile([32, 512], FR, tag="ps")
            kpsI = ps.tile([32, 512], FR, tag="ps")
            for ech in range(4):
                nc.tensor.transpose(kpsR[0:32, ech * 128:ech * 128 + 128],
                                    qkf[:, ech, b, 64:96], ident[:])
                nc.tensor.transpose(kpsI[0:32, ech * 128:ech * 128 + 128],
                                    qkf[:, ech, b, 96:128], ident[:])
            kfTR = wrk.tile([32, 512], FR, tag="kfTR")
            kfTI = wrk.tile([32, 512], FR, tag="kfTI")
            nc.vector.tensor_copy(kfTR[:], kpsR[0:32, :])
            nc.vector.tensor_copy(kfTI[:], kpsI[0:32, :])
            vps = ps.tile([128, 512], F32, tag="ps")
            for ech in range(4):
                nc.tensor.matmul(vps[:, ech * 64:ech * 64 + 64],
                                 kfTR[:, ech * 128:ech * 128 + 128],
                                 U1[:, b, :], start=True, stop=False)
                nc.tensor.matmul(vps[:, ech * 64:ech * 64 + 64],
                                 kfTI[:, ech * 128:ech * 128 + 128],
                                 U2[:, b, :], start=False, stop=True)
            vsb = wrk.tile([128, 4, 64], FR, tag="kfTI2")
            nc.vector.tensor_copy(
                vsb[:], vps[:, 0:256].rearrange("p (c m) -> p c m", c=4))
            vTp = ps.tile([64, 512], FR, tag="ps")
            for ech in range(4):
                nc.tensor.transpose(vTp[0:64, ech * 128:ech * 128 + 128],
                                    vsb[:, ech, :], ident[:])
            vT_sb = wrk.tile([64, 512], FR, tag="vT")
            nc.vector.tensor_copy(vT_sb[:], vTp[0:64, :])
            nc.sync.dma_start(out=vq_d[b], in_=vT_sb[:])

        # =================== stage D: AllToAll + mode-sharded einsum =======
        # exchange 1: route each core's xqkv columns to the mode owner
        grp = [list(range(NCORES))]
        nc.sync.dma_start(
            out=vq_snd[:],
            in_=vq_d.rearrange("b (k j m) e -> j b k m e", k=2, j=NCORES,
                               m=MLOC))
        nc.gpsimd.collective_compute(
            "AllToAll", ALU.bypass, replica_groups=grp,
            ins=[vq_snd.opt()], outs=[vq_rcv.opt()])
        # xqgT: partition p = kind*32 + b_global, free (m_local, e)
        xqgT = wrk.tile([64, MLOC, D], FR, tag="mid")
        for k in range(2):
            nc.sync.dma_start(
                out=xqgT[k * 32:k * 32 + 32, :, :],
                in_=vq_rcv[:, :, k, :, :].rearrange("s b m e -> (s b) m e"))
        # transpose back to (e-part, cols=(kind, b_global)) per (m', ech)
        xqa = wrk.tile([128, 4, MLOC, 64], FR, tag="mid2")
        xqa2 = wrk.tile([128, 4, MLOC, 64], FR, tag="qkve2")
        for ml in range(MLOC):
            xp = ps.tile([128, 512], FR, tag="ps")
            for ech in range(4):
                nc.tensor.transpose(xp[:, ech * 64:ech * 64 + 64],
                                    xqgT[0:64, ml, ech * 128:ech * 128 + 128],
                                    ident[0:64, 0:64])
            nc.vector.tensor_copy(
                xqa[:, :, ml, :],
                xp[:, 0:256].rearrange("p (c m) -> p c m", c=4))
        nc.scalar.activation(xqa2[:, :, :, 0:32], xqa[:, :, :, 32:64],
                             AF.Copy, scale=-1.0)
        nc.vector.tensor_copy(xqa2[:, :, :, 32:64], xqa[:, :, :, 0:32])
        _nmodes = 0 if os.environ.get("BK_SKIP_D") else MLOC
        with tc.tile_pool(name="pse", bufs=2, space="PSUM") as pse:
            for ml in range(_nmodes):
                pm = pse.tile([64, 512], F32, tag="pm")
                for ech in range(4):
                    wR = wpool.tile([128, 512], FR, tag="wt")
                    nc.sync.dma_start(
                        out=wR[:],
                        in_=dram["wslab"][ml, 0, ech * 128:ech * 128 + 128, :])
                    wI = wpool.tile([128, 512], FR, tag="wt")
                    nc.sync.dma_start(
                        out=wI[:],
                        in_=dram["wslab"][ml, 1, ech * 128:ech * 128 + 128, :])
                    nc.tensor.matmul(pm[:], xqa[:, ech, ml, :], wR,
                                     start=(ech == 0), stop=False)
                    nc.tensor.matmul(pm[:], xqa2[:, ech, ml, :], wI,
                                     start=False, stop=(ech == 3))
                xw_sb = outp.tile([64, 512], FR, tag="mid3")
                nc.vector.tensor_copy(xw_sb[:], pm[:])
                nc.sync.dma_start(out=xwm_d[ml], in_=xw_sb[:])
        # exchange 2: route per-mode results back to batch owners
        nc.sync.dma_start(
            out=xw_snd[:],
            in_=xwm_d.rearrange("m (r j bl) o -> j m r bl o", r=2, j=NCORES,
                                bl=BPC))
        nc.gpsimd.collective_compute(
            "AllToAll", ALU.bypass, replica_groups=grp,
            ins=[xw_snd.opt()], outs=[xw_rcv.opt()])

        # =================== stage E: irfft, Wo, MA, convs, BN =============
        _bpce = 0 if os.environ.get("BK_SKIP_E") else BPC
        for b in range(_bpce):
            XXT = wrk.tile([64, 512], FR, tag="xxt")
            for r in range(2):
                nc.sync.dma_start(
                    out=XXT[r * 32:r * 32 + 32, :],
                    in_=xw_rcv[:, :, r, b, :].rearrange("s m o -> (s m) o"))
            fre = big.tile([128, 4, L], FR, tag="e2")
            for och in range(4):
                for lh in range(2):
                    fp = ps2.tile([128, 512], F32, tag="ps2")
                    nc.tensor.matmul(fp[:],
                                     XXT[:, och * 128:och * 128 + 128],
                                     gtab[:, lh * 512:lh * 512 + 512],
                                     start=True, stop=True)
                    nc.vector.tensor_copy(fre[:, och, lh * 512:lh * 512 + 512],
                                          fp[:])
            # reload x and transpose it to (d-part, l) for the residual
            xe = big.tile([128, 8, D], FR, tag="xx")
            nc.sync.dma_start(
                out=xe[:],
                in_=dram["xs"][b].rearrange("(t p) d -> p t d", p=128))
            xT = big.tile([128, 4, L], FR, tag="e4")
            for dch in range(4):
                for lh in range(2):
                    tp0 = ps.tile([128, 512], FR, tag="ps")
                    for lq in range(4):
                        lt = lh * 4 + lq
                        nc.tensor.transpose(
                            tp0[:, lq * 128:lq * 128 + 128],
                            xe[:, lt, dch * 128:dch * 128 + 128], ident[:])
                    if (dch + lh) % 2 == 0:
                        nc.vector.tensor_copy(
                            xT[:, dch, lh * 512:lh * 512 + 512], tp0[:])
                    else:
                        nc.scalar.copy(
                            xT[:, dch, lh * 512:lh * 512 + 512], tp0[:])
            u_s = big.tile([128, 4, L], FR, tag="e5")
            for dch in range(4):
                for lh in range(2):
                    up = ps.tile([128, 512], F32, tag="ps")
                    for och in range(4):
                        nc.tensor.matmul(
                            up[:], wot[:, och, dch * 128:dch * 128 + 128],
                            fre[:, och, lh * 512:lh * 512 + 512],
                            start=(och == 0), stop=False)
                    nc.tensor.matmul(
                        up[:], ident[:],
                        xT[:, dch, lh * 512:lh * 512 + 512],
                        start=False, stop=True)
                    nc.scalar.copy(u_s[:, dch, lh * 512:lh * 512 + 512], up[:])
            cs = big.tile([128, 4, L], FR, tag="e4")
            for dch in range(4):
                nc.vector.tensor_tensor_scan(
                    cs[:, dch, :], u_s[:, dch, :], u_s[:, dch, :], 0.0,
                    ALU.add, ALU.bypass)
            tmov = big.tile([128, 4, L], FR, tag="e1")
            ef = wrk.tile([128, 65], FR, tag="ef")
            ee_ = wrk.tile([128, 63], FR, tag="ee")
            e2_ = wrk.tile([128, 63], FR, tag="e2s")
            for dch in range(4):
                tt(tmov[:, dch, 65:961], cs[:, dch, 128:1024],
                   cs[:, dch, 0:896], ALU.subtract)
                tt(ef[:], u_s[:, dch, 0:1].to_broadcast([128, 65]), coefF[:],
                   ALU.mult)
                tt(tmov[:, dch, 0:65], cs[:, dch, 63:128], ef[:], ALU.add)
                tt(ee_[:], u_s[:, dch, 1023:1024].to_broadcast([128, 63]),
                   coefE[:], ALU.mult)
                tt(e2_[:], cs[:, dch, 1023:1024].to_broadcast([128, 63]),
                   cs[:, dch, 896:959], ALU.subtract)
                tt(tmov[:, dch, 961:1024], ee_[:], e2_[:], ALU.add)
            # x_dec = u - mov(u), computed in place into u_s
            nc.scalar.activation(tmov[:], tmov[:], AF.Copy, scale=1.0 / 128)
            tt(u_s[:], u_s[:], tmov[:], ALU.subtract)
            xd = u_s
            y1g = big.tile([128, 4, L], FR, tag="e2")
            for och in range(4):
                for lh in range(2):
                    cp = ps2.tile([128, 512], F32, tag="ps2")
                    for dch in range(4):
                        nc.tensor.matmul(
                            cp[:], w1t[:, dch, och * 128:och * 128 + 128],
                            xd[:, dch, lh * 512:lh * 512 + 512],
                            start=(dch == 0), stop=(dch == 3))
                    yslc = y1g[:, och, lh * 512:lh * 512 + 512]
                    if not SIM_GELU:
                        nc.scalar.activation(yslc, cp[:], AF.Gelu)
                    else:
                        # CoreSim has no Gelu LUT: tanh-approx stand-in
                        y1c = wrk.tile([128, 512], F32, tag="gel1")
                        nc.scalar.copy(y1c[:], cp[:])
                        sqt = wrk.tile([128, 512], F32, tag="gel2")
                        nc.scalar.activation(sqt[:], y1c[:], AF.Square)
                        tt(sqt[:], sqt[:], y1c[:], ALU.mult)
                        nc.vector.tensor_scalar(sqt[:], sqt[:], 0.044715,
                                                None, ALU.mult)
                        tt(sqt[:], sqt[:], y1c[:], ALU.add)
                        nc.vector.tensor_scalar(sqt[:], sqt[:],
                                                0.7978845608028654,
                                                None, ALU.mult)
                        nc.scalar.activation(sqt[:], sqt[:], AF.Tanh)
                        nc.vector.tensor_scalar(sqt[:], sqt[:], 0.5, 0.5,
                                                ALU.mult, ALU.add)
                        tt(yslc, y1c[:], sqt[:], ALU.mult)
            res = big.tile([128, 4, L], FR, tag="e1")
            for dch in range(4):
                for lh in range(2):
                    rp = ps.tile([128, 512], F32, tag="ps")
                    for och in range(4):
                        nc.tensor.matmul(
                            rp[:], w2t[:, och, dch * 128:dch * 128 + 128],
                            y1g[:, och, lh * 512:lh * 512 + 512],
                            start=(och == 0), stop=False)
                    nc.tensor.matmul(
                        rp[:], ident[:],
                        xd[:, dch, lh * 512:lh * 512 + 512],
                        start=False, stop=True)
                    nc.vector.tensor_copy(
                        res[:, dch, lh * 512:lh * 512 + 512], rp[:])
            for lt in range(8):
                tp = ps.tile([128, 512], FR, tag="ps")
                for dch in range(4):
                    nc.tensor.transpose(
                        tp[:, dch * 128:dch * 128 + 128],
                        res[:, dch, lt * 128:lt * 128 + 128], ident[:])
                ob = outp.tile([128, 512], F32, tag="ob")
                nc.scalar.activation(ob[:], tp[:], AF.Identity,
                                     bias=c0l[:, lt:lt + 1],
                                     scale=c1l[:, lt:lt + 1])
                nc.sync.dma_start(out=out_d[b, lt * 128:lt * 128 + 128, :],
                                  in_=ob[:])

    nc.compile()
    return nc


_CACHE = {}


def _get_nc():
    if "nc" not in _CACHE:
        _CACHE["nc"] = _build()
    return _CACHE["nc"]


def _host_inputs(inputs):
    x = np.ascontiguousarray(inputs["x"], dtype=np.float32)
    ftab, gtab, ident, coefF, coefE = _tables()
    wr = np.asarray(inputs["w_real"], dtype=np.float32)[0]   # (E, O, MODES)
    wi = np.asarray(inputs["w_imag"], dtype=np.float32)[0]
    wslab = np.ascontiguousarray(
        np.stack([wr.transpose(2, 0, 1), wi.transpose(2, 0, 1)], axis=1))
    bn = [np.asarray(inputs[k], dtype=np.float32)
          for k in ("bn_gamma", "bn_beta", "bn_mean", "bn_var")]
    bnt = np.ascontiguousarray(
        np.stack(bn, -1).reshape(8, 128, 4).transpose(1, 0, 2))
    bq = np.asarray(inputs["bq"], dtype=np.float32)
    bk = np.asarray(inputs["bk"], dtype=np.float32)
    bqkt = np.ascontiguousarray(
        np.stack([bq.reshape(4, 128).T, bk.reshape(4, 128).T], -1))
    com = {
        "wqt": _t128(np.asarray(inputs["Wq"], np.float32).T),
        "wkt": _t128(np.asarray(inputs["Wk"], np.float32).T),
        "wot": _t128(np.asarray(inputs["Wo"], np.float32).T),
        "w1t": _t128(np.asarray(inputs["conv1_w"], np.float32).T),
        "w2t": _t128(np.asarray(inputs["conv2_w"], np.float32).T),
        "ftab": ftab, "gtab": gtab, "identt": ident,
        "coefF": coefF, "coefE": coefE, "bnt": bnt, "bqkt": bqkt,
    }
    mloc = MODES // NCORES
    maps = []
    for c in range(NCORES):
        m = dict(com)
        m["xs"] = np.ascontiguousarray(x[c * BPC:(c + 1) * BPC])
        m["wslab"] = np.ascontiguousarray(wslab[c * mloc:(c + 1) * mloc])
        maps.append(m)
    return maps


def kernel(**inputs):
    nc = _get_nc()
    in_maps = _host_inputs(inputs)
    trace = bool(int(os.environ.get("BK_TRACE", "0")))
    res = run_bass_kernel_spmd(nc, in_maps, core_ids=list(range(NCORES)),
                               trace=trace)
    if trace and res.exec_time_ns is not None:
        print(f"HW exec time: {res.exec_time_ns} ns")
        _CACHE["exec_time_ns"] = res.exec_time_ns
    out = np.concatenate([res.results[c]["out"] for c in range(NCORES)], 0)
    return out.astype(np.float32)

